# revision 33
# baseline (speedup 1.0000x reference)
"""DKVMN memory-update kernel for Trainium2 (8 NeuronCores, data-parallel batch).

Per-core computation (Bc = B/8 batch rows):
  qk    = tanh(q @ Wq.T + bq)                  [Bc, KD]
  corr  = softmax(qk @ km.T, axis=1)           [Bc, M]
  rc    = einsum('bm,bmv->bv', corr, vmm)      [Bc, VD]
  erase = sigmoid(ve @ We.T + be)              [Bc, VD]
  add   = tanh(ve @ Wa.T + ba)                 [Bc, VD]
  upd   = vmm * (1 - corr[:,:,None]*erase[:,None,:]) + corr[:,:,None]*add[:,None,:]

Strategy: the 1 GiB vmm stream is the bottleneck (memory regime; HBM floor
~0.76 ms/core for 271 MB of traffic at 358 GB/s).  Tiles are laid out
[m-in-chunk(128 partitions), (m-chunk, v)] so the rank-1 fields are produced by
tiny TensorE outer-product matmuls: per (b, m-chunk) ONE K=1 fp32r matmul with
rhs = [-erase | add] writes [-w(x)e | w(x)a] into one PSUM bank (fp32r streams
1 cyc/row vs 4 for fp32; its ~12-bit rounding only touches the rank-1 factors,
keeping upd error ~1e-5).  ScalarE turns -w(x)e into 1-w(x)e (Identity,
bias=1), leaving VectorE exactly 2 tensor_tensor ops per element:
X = vmm*P1s ; upd = X + P2.  read_content rides along as accumulating K=128
fp32 mat-vec matmuls on TensorE (kept full precision).  Small linears/softmax
are negligible.  Cost model: makespan 0.805 ms/core, DMA-bound (PE 0.67,
DVE 0.64); paired wall-clock on HW measured ~0.78-0.88 ms/exec.

PE matmul operand APs must sit at partition base 0/32/64, so the per-b rows
[corr | -erase | add] bounce through DRAM scratch after the front phase and
are re-loaded per pair of b at partitions {0, 64}.
"""

import numpy as np

import concourse.bass as bass
import concourse.tile as tile
from concourse import bacc, mybir
from concourse.masks import make_identity
from concourse import bass_utils

F32 = mybir.dt.float32
F32R = mybir.dt.float32r
AF = mybir.ActivationFunctionType

# Problem constants (hardcoded per spec nn_DKVMN_62302795596377)
B_FULL, M, KD, VD = 2048, 512, 128, 256
NCORES = 8
MC = M // 128  # m-chunks


def build(Bc: int, Ng: int = 4, rep: int = 1) -> bass.Bass:
    """Build the per-core Bass module for a per-core batch of Bc rows."""
    assert Bc % Ng == 0
    assert Bc % 128 == 0 or Bc <= 128
    n_bch = max(1, Bc // 128)
    bch = min(Bc, 128)  # rows per batch chunk
    KC = VD // 128      # contraction chunks for the VD-wide linears

    nc = bacc.Bacc("TRN2")

    # ---- DRAM I/O (per-core shapes; host pre-transposes the small tensors) ----
    qT_d = nc.dram_tensor("qT", [KD, Bc], F32, kind="ExternalInput")
    veT_d = nc.dram_tensor("veT", [128, KC, Bc], F32, kind="ExternalInput")
    vmm_d = nc.dram_tensor("vmm", [Bc, M, VD], F32, kind="ExternalInput")
    kmT_d = nc.dram_tensor("kmT", [KD, M], F32, kind="ExternalInput")
    WqT_d = nc.dram_tensor("WqT", [KD, KD], F32, kind="ExternalInput")
    WeT_d = nc.dram_tensor("WeT", [128, KC, VD], F32, kind="ExternalInput")
    WaT_d = nc.dram_tensor("WaT", [128, KC, VD], F32, kind="ExternalInput")
    bq_d = nc.dram_tensor("bq", [KD, 1], F32, kind="ExternalInput")
    be_d = nc.dram_tensor("be", [1, VD], F32, kind="ExternalInput")
    ba_d = nc.dram_tensor("ba", [1, VD], F32, kind="ExternalInput")

    rc_d = nc.dram_tensor("rc", [Bc, VD], F32, kind="ExternalOutput")
    upd_d = nc.dram_tensor("upd", [Bc, M, VD], F32, kind="ExternalOutput")

    with tile.TileContext(nc) as tc:
        with (
            tc.tile_pool(name="persist", bufs=1) as persist,
            tc.tile_pool(name="scr", bufs=1, space="DRAM") as scr,
        ):
            identity = persist.tile([128, 128], F32)
            make_identity(nc, identity)
            # all-ones on every partition: rows are sliced at whatever base
            # partition the outer-product matmuls use ({0,32,64,96})
            ones = persist.tile([128, 512], F32)
            nc.vector.memset(ones, 1.0)

            qT = persist.tile([KD, Bc], F32)
            veT = persist.tile([128, KC, Bc], F32)
            kmT = persist.tile([KD, M], F32)
            WqT = persist.tile([KD, KD], F32)
            WeT = persist.tile([128, KC, VD], F32)
            WaT = persist.tile([128, KC, VD], F32)
            bq = persist.tile([KD, 1], F32)
            be = persist.tile([1, VD], F32)
            ba = persist.tile([1, VD], F32)
            for t, d in ((qT, qT_d), (veT, veT_d), (kmT, kmT_d), (WqT, WqT_d),
                         (WeT, WeT_d), (WaT, WaT_d), (bq, bq_d), (be, be_d),
                         (ba, ba_d)):
                nc.sync.dma_start(out=t, in_=d[:])

            # computed small tensors
            qkT = persist.tile([KD, Bc], F32)              # [j, b]
            corr = persist.tile([128, n_bch, M], F32)      # [b_in_chunk, bc, m]
            corrT = persist.tile([128, MC, Bc], F32)       # [m_in_chunk, mc, b]
            erase_neg = persist.tile([128, n_bch, VD], F32)
            addt = persist.tile([128, n_bch, VD], F32)

            # DRAM scratch for per-b rows (reloaded at partition 0 later).
            # One row per b: [corr(512) | -erase(256) | add(256)].
            # float32r end-to-end: the PE consumes these in fp32r matmuls and
            # the verifier requires the producer chain to carry the tag.
            rows_scr = scr.tile([Bc, M + 2 * VD], F32R)

            # ---------------- front phase: small linears + softmax ----------------
            with (
                tc.tile_pool(name="fps", bufs=2, space="PSUM") as fps,
                tc.tile_pool(name="fsb", bufs=2) as fsb,
            ):
                # qkT = tanh(WqT.T @ qT + bq)   [j, b]
                qk_ps = fps.tile([KD, Bc], F32, tag="fp")
                nc.tensor.matmul(qk_ps, lhsT=WqT, rhs=qT, start=True, stop=True)
                nc.scalar.activation(out=qkT, in_=qk_ps, func=AF.Tanh, bias=bq)

                for bc in range(n_bch):
                    bsl = slice(bc * 128, bc * 128 + bch)
                    # logits [b, m] for this batch chunk
                    lg = fps.tile([128, M], F32, tag="fp")
                    nc.tensor.matmul(
                        lg[:bch], lhsT=qkT[:, bsl], rhs=kmT, start=True, stop=True
                    )
                    mx = fsb.tile([128, 1], F32, tag="mx")
                    nc.vector.tensor_reduce(
                        mx[:bch], lg[:bch], mybir.AxisListType.X, mybir.AluOpType.max
                    )
                    nmx = fsb.tile([128, 1], F32, tag="nmx")
                    nc.vector.tensor_scalar_mul(nmx[:bch], mx[:bch], -1.0)
                    ssum = fsb.tile([128, 1], F32, tag="ssum")
                    nc.scalar.activation(
                        out=corr[:bch, bc, :], in_=lg[:bch], func=AF.Exp,
                        bias=nmx[:bch], accum_out=ssum[:bch],
                    )
                    rec = fsb.tile([128, 1], F32, tag="rec")
                    nc.vector.reciprocal(rec[:bch], ssum[:bch])
                    nc.vector.tensor_scalar_mul(
                        corr[:bch, bc, :], corr[:bch, bc, :], rec[:bch]
                    )
                    nc.sync.dma_start(
                        out=rows_scr[bsl, 0:M],
                        in_=corr[:bch, bc, :].bitcast(F32R),
                    )

                    # erase (negated) and add  [b_in_chunk, v]
                    for dst, wT, bias, fn, col in (
                        (erase_neg, WeT, be, AF.Sigmoid, M),
                        (addt, WaT, ba, AF.Tanh, M + VD),
                    ):
                        ps = fps.tile([128, VD], F32, tag="fp")
                        for kc in range(KC):
                            nc.tensor.matmul(
                                ps[:bch], lhsT=veT[:, kc, bsl], rhs=wT[:, kc, :],
                                start=(kc == 0), stop=False,
                            )
                        nc.tensor.matmul(
                            ps[:bch], lhsT=ones[0:1, :bch], rhs=bias,
                            start=False, stop=True,
                        )
                        nc.scalar.activation(
                            out=dst[:bch, bc, :], in_=ps[:bch], func=fn
                        )
                        if fn == AF.Sigmoid:
                            nc.vector.tensor_scalar_mul(
                                dst[:bch, bc, :], dst[:bch, bc, :], -1.0
                            )
                        nc.sync.dma_start(
                            out=rows_scr[bsl, col:col + VD],
                            in_=dst[:bch, bc, :].bitcast(F32R),
                        )

                    # corr transposes -> corrT [m_in_chunk, mc, b]
                    for mc in range(MC):
                        tr = fps.tile([128, 128], F32, tag="tr")
                        nc.tensor.transpose(
                            tr[:, :bch],
                            corr[:bch, bc, mc * 128:(mc + 1) * 128],
                            identity[:bch, :bch],
                        )
                        nc.scalar.copy(corrT[:, mc, bsl], tr[:, :bch])

            # ---------------- main streaming loop ----------------
            with (
                tc.tile_pool(name="vin_p", bufs=3) as vin_p,
                tc.tile_pool(name="uout_p", bufs=3) as uout_p,
                tc.tile_pool(name="x_p", bufs=8) as x_p,
                tc.tile_pool(name="p1s_p", bufs=8) as p1s_p,
                tc.tile_pool(name="rows_p", bufs=6) as rows_p,
                tc.tile_pool(name="rcsb_p", bufs=3) as rcsb_p,
                tc.tile_pool(name="mps", bufs=3, space="PSUM") as mps,
                tc.tile_pool(name="rcps_p", bufs=2, space="PSUM") as rcps_p,
            ):
                def pair_ap(t, n_free):
                    """AP over tile `t` selecting rows at partitions 0/64.

                    AP steps are linearized elements: partition p starts at
                    p * partition_pitch, so partitions {0,64} = step 64*pitch.
                    """
                    pitch = t[:].ap[0][0]
                    return bass.AP(
                        tensor=t.tensor, offset=t.offset,
                        ap=[[64 * pitch, 2], [1, n_free]],
                    )

                if rep > 1:
                    # benchmark-only: loop the whole stream `rep` times
                    _repctx = tc.For_i(0, rep, 1)
                    _repctx.__enter__()
                for g in range(Bc // Ng):
                    gsl = slice(g * Ng, (g + 1) * Ng)
                    vin = vin_p.tile([128, Ng, MC, VD], F32, tag="vin")
                    nc.sync.dma_start(
                        out=vin,
                        in_=vmm_d[gsl].rearrange("ib (mc p) v -> p ib mc v", p=128),
                    )
                    uout = uout_p.tile([128, Ng, MC, VD], F32, tag="uout")

                    for half in range(Ng // 2):
                        b0 = g * Ng + half * 2
                        # 2 consecutive b's rows staged at partitions 0/64
                        rows = rows_p.tile([128, M + 2 * VD], F32R, tag="rows")
                        nc.sync.dma_start(
                            out=pair_ap(rows, M + 2 * VD),
                            in_=rows_scr[b0:b0 + 2, :],
                        )
                        rc_ps = rcps_p.tile([1, 2, VD], F32, tag="rcps")
                        rc_sb = rcsb_p.tile([1, 2, VD], F32, tag="rcsb")

                        for i in range(2):
                            ib = half * 2 + i
                            b = g * Ng + ib
                            pb = 64 * i
                            ea_row = rows[pb:pb + 1, M:M + 2 * VD]

                            for mh in range(MC // 2):
                                # p12[:, j, 0, :] = -w(x)e ; p12[:, j, 1, :] = w(x)a
                                p12 = mps.tile([128, 2, 2, VD], F32, tag="p12")
                                for j, mc in enumerate((2 * mh, 2 * mh + 1)):
                                    w_row = rows[pb:pb + 1,
                                                 mc * 128:(mc + 1) * 128]
                                    nc.tensor.matmul(
                                        p12[:, j, :, :], lhsT=w_row,
                                        rhs=ea_row,
                                        start=True, stop=True,
                                    )
                                # +1 on ScalarE: p1s = (-w(x)e) + 1
                                p1s = p1s_p.tile([128, 2, VD], F32, tag="p1s")
                                nc.scalar.activation(
                                    out=p1s, in_=p12[:, :, 0, :],
                                    func=AF.Identity, bias=1.0,
                                )
                                x = x_p.tile([128, 2, VD], F32, tag="x")
                                vslice = vin[:, ib, 2 * mh:2 * mh + 2, :]
                                nc.vector.tensor_mul(x, vslice, p1s)
                                nc.vector.tensor_add(
                                    uout[:, ib, 2 * mh:2 * mh + 2, :],
                                    x, p12[:, :, 1, :],
                                )

                            for mc in range(MC):
                                nc.tensor.matmul(
                                    rc_ps[0:1, i, :],
                                    lhsT=corrT[:, mc, b:b + 1],
                                    rhs=vin[:, ib, mc, :],
                                    start=(mc == 0), stop=(mc == MC - 1),
                                )
                        nc.scalar.copy(rc_sb, rc_ps)
                        nc.sync.dma_start(
                            out=rc_d[b0:b0 + 2, :].rearrange(
                                "(o ib) v -> o ib v", o=1
                            ),
                            in_=rc_sb,
                        )

                    nc.scalar.dma_start(
                        out=upd_d[gsl].rearrange("ib (mc p) v -> p ib mc v", p=128),
                        in_=uout,
                    )
                if rep > 1:
                    _repctx.__exit__(None, None, None)
    nc.compile()
    return nc


_BUILT: dict[int, bass.Bass] = {}
TRACE = False           # set True (e.g. from test.py) to capture an NTFF trace
LAST_RESULT = [None]    # BassKernelResults of the most recent run


def _get_built(Bc: int) -> bass.Bass:
    if Bc not in _BUILT:
        _BUILT[Bc] = build(Bc)
    return _BUILT[Bc]


def _in_map(c: int, Bc: int, q, ve, vmm, kmT, WqT, WeT, WaT, bq2, be2, ba2):
    sl = slice(c * Bc, (c + 1) * Bc)
    qT = np.ascontiguousarray(q[sl].T)
    veT = np.ascontiguousarray(
        ve[sl].T.reshape(VD // 128, 128, Bc).transpose(1, 0, 2)
    )
    return dict(
        qT=qT, veT=veT, vmm=np.ascontiguousarray(vmm[sl]), kmT=kmT,
        WqT=WqT, WeT=WeT, WaT=WaT, bq=bq2, be=be2, ba=ba2,
    )


def kernel(query_embedded, value_embedded, value_memory_matrix,
           key_memory, Wq, bq, We, be, Wa, ba):
    q = np.asarray(query_embedded, dtype=np.float32)
    ve = np.asarray(value_embedded, dtype=np.float32)
    vmm = np.asarray(value_memory_matrix, dtype=np.float32)
    km = np.asarray(key_memory, dtype=np.float32)
    Wq = np.asarray(Wq, dtype=np.float32)
    bq = np.asarray(bq, dtype=np.float32)
    We = np.asarray(We, dtype=np.float32)
    be = np.asarray(be, dtype=np.float32)
    Wa = np.asarray(Wa, dtype=np.float32)
    ba = np.asarray(ba, dtype=np.float32)

    Bc = q.shape[0] // NCORES
    nc = _get_built(Bc)

    kmT = np.ascontiguousarray(km.T)
    WqT = np.ascontiguousarray(Wq.T)
    WeT = np.ascontiguousarray(We.T.reshape(VD // 128, 128, VD).transpose(1, 0, 2))
    WaT = np.ascontiguousarray(Wa.T.reshape(VD // 128, 128, VD).transpose(1, 0, 2))
    bq2 = np.ascontiguousarray(bq.reshape(KD, 1))
    be2 = np.ascontiguousarray(be.reshape(1, VD))
    ba2 = np.ascontiguousarray(ba.reshape(1, VD))

    in_maps = [
        _in_map(c, Bc, q, ve, vmm, kmT, WqT, WeT, WaT, bq2, be2, ba2)
        for c in range(NCORES)
    ]
    res = bass_utils.run_bass_kernel_spmd(
        nc, in_maps, core_ids=list(range(NCORES)), trace=TRACE
    )
    LAST_RESULT[0] = res
    rc = np.concatenate([r["rc"] for r in res.results], axis=0)
    upd = np.concatenate([r["upd"] for r in res.results], axis=0)
    return rc, upd
